# revision 2
# baseline (speedup 1.0000x reference)
"""Trainium2 Bass kernel for nn_EmbedMatcher (GNN message passing).

Strategy: data-parallel over B=1024 across 8 cores (128 rows each); the
200001x128 symbol table is replicated per core. The neighbor gather+sum is
the memory-bound phase: instead of one indirect DMA per neighbor column
(400 instructions x ~1us SWDGE overhead each), we issue 8 wide indirect
DMAs, each gathering G=50 rows per partition ([128, 50] offset AP ->
[128, 50*128] out tile); the second chunk of each chain folds into the
first via compute_op=add. A strided tensor_reduce then collapses the 50
gathered rows per partition. The GCN linear is algebraically reordered:
sum_k (concat @ W^T) == [rel_sum|ent_sum] @ W^T. LSTM step-0 gates depend
only on the query row, so they are computed during the gather phase.
support_g is AllGathered so each core runs the batch-coupled attention on
its own 128 query rows.
"""
import numpy as np

from concourse import bass, bacc, mybir
import concourse.tile as tile
from concourse.bass_utils import run_bass_kernel_spmd

P = 128            # batch rows per core
D = 128            # embed dim
K = 200            # neighbors
NCORES = 8
NROWS = 200001     # symbol table rows (incl. padding row)
STEPS = 4
G = 50             # index columns per wide gather instruction
F32 = mybir.dt.float32
I32 = mybir.dt.int32

_CACHE = {}

# weight pack layout: name -> (col_start, ncols), all [128, n] f32 blocks
_WSPECS = [
    ("wrT", D), ("weT", D), ("gcnb", D),
    ("p1wT", 2 * D), ("p1b", 2),
    ("p2wTa", D), ("p2wTb", D), ("p2b", D),
    ("lna", D), ("lnb", D),
    ("wihT", 8 * D), ("whhTa", 8 * D), ("whhTb", 8 * D), ("gbias", 8 * D),
]
_WOFF = {}
_c = 0
for _n, _w in _WSPECS:
    _WOFF[_n] = (_c, _w)
    _c += _w
WCOLS = _c


def _build():
    nc = bacc.Bacc("TRN2", target_bir_lowering=False, debug=False,
                   enable_asserts=True, num_devices=NCORES)
    ap = {}
    ap["table"] = nc.dram_tensor("table", [NROWS, D], F32,
                                 kind="ExternalInput").ap()
    ap["idx"] = nc.dram_tensor("idx", [P, 2 * K], I32,
                               kind="ExternalInput").ap()
    ap["qidx"] = nc.dram_tensor("qidx", [P, 1], I32,
                                kind="ExternalInput").ap()
    ap["wpack"] = nc.dram_tensor("wpack", [P, WCOLS], F32,
                                 kind="ExternalInput").ap()
    out_d = nc.dram_tensor("out", [P, 1], F32, kind="ExternalOutput").ap()

    from concourse.masks import make_identity
    AX = mybir.AxisListType.X
    OP = mybir.AluOpType
    ACT = mybir.ActivationFunctionType

    with tile.TileContext(nc, num_cores=NCORES) as tc:
        with tc.tile_pool(name="sb", bufs=1) as sb, \
             tc.tile_pool(name="ps", bufs=2, space="PSUM") as ps, \
             tc.tile_pool(name="pst", bufs=2, space="PSUM") as pst, \
             tc.tile_pool(name="dram", bufs=1, space="DRAM") as dram:

            ident = sb.tile([P, P], F32)
            make_identity(nc, ident[:])

            # ---- load inputs to SBUF
            idx_sb = sb.tile([P, 2 * K], I32)
            nc.sync.dma_start(out=idx_sb[:], in_=ap["idx"][:])
            qidx_sb = sb.tile([P, 1], I32)
            nc.sync.dma_start(out=qidx_sb[:], in_=ap["qidx"][:])
            wsb = sb.tile([P, WCOLS], F32)
            nc.sync.dma_start(out=wsb[:], in_=ap["wpack"][:])

            def w(name):
                c0, n = _WOFF[name]
                return wsb[:, c0:c0 + n]

            # ---- query gather first (tiny) so step-0 precompute can start
            q_sb = sb.tile([P, D], F32)
            nc.gpsimd.indirect_dma_start(
                out=q_sb[:], out_offset=None, in_=ap["table"][:],
                in_offset=bass.IndirectOffsetOnAxis(ap=qidx_sb[:, 0:1], axis=0))

            # ---- neighbor gathers: 8 wide indirect DMAs (4 chains x 2)
            # chain i covers idx cols [i*100, i*100+100); chunk 2 accumulates
            gbuf = [sb.tile([P, G * D], F32, name=f"gbuf{i}") for i in range(4)]
            for ch in range(2):
                for i in range(4):
                    col0 = i * 2 * G + ch * G
                    nc.gpsimd.indirect_dma_start(
                        out=gbuf[i][:], out_offset=None,
                        in_=ap["table"][:],
                        in_offset=bass.IndirectOffsetOnAxis(
                            ap=idx_sb[:, col0:col0 + G], axis=0),
                        compute_op=(OP.bypass if ch == 0 else OP.add))

            def transpose_to(dst_sb, src_ap, nm):
                tp = pst.tile([P, P], F32, name=f"tp_{nm}", tag="tp")
                nc.tensor.transpose(out=tp[:], in_=src_ap, identity=ident[:])
                nc.vector.tensor_copy(out=dst_sb, in_=tp[:])

            # ---- LSTM step-0 precompute (depends only on q + weights);
            # runs on PE/ACT/DVE while the gathers stream.
            qT = sb.tile([P, P], F32)
            transpose_to(qT[:], q_sb[:], "q")
            gts_q = sb.tile([P, 8 * D], F32)   # q @ w_ih.T + b_ih + b_hh
            for j in range(2):
                gp = ps.tile([P, 512], F32, name=f"gq{j}", tag="gates")
                sl = slice(512 * j, 512 * (j + 1))
                nc.tensor.matmul(out=gp[:], lhsT=qT[:], rhs=w("wihT")[:, sl],
                                 start=True, stop=True)
                nc.vector.tensor_add(out=gts_q[:, sl], in0=gp[:],
                                     in1=w("gbias")[:, sl])
            c_sb = sb.tile([P, 2 * D], F32)
            si = sb.tile([P, 2 * D], F32)
            sf = sb.tile([P, 2 * D], F32)
            tg = sb.tile([P, 2 * D], F32)
            so = sb.tile([P, D], F32)
            tch = sb.tile([P, D], F32)
            ho = sb.tile([P, D], F32)
            hoT = sb.tile([P, P], F32)
            nc.scalar.activation(out=si[:], in_=gts_q[:, 0:256], func=ACT.Sigmoid)
            nc.scalar.activation(out=tg[:], in_=gts_q[:, 512:768], func=ACT.Tanh)
            nc.scalar.activation(out=so[:], in_=gts_q[:, 768:896], func=ACT.Sigmoid)
            nc.vector.tensor_tensor(out=c_sb[:], in0=si[:], in1=tg[:], op=OP.mult)
            nc.scalar.activation(out=tch[:], in_=c_sb[:, 0:D], func=ACT.Tanh)
            nc.vector.tensor_tensor(out=tch[:], in0=so[:], in1=tch[:], op=OP.mult)
            nc.vector.tensor_add(out=ho[:], in0=q_sb[:], in1=tch[:])
            transpose_to(hoT[:], ho[:], "ho0")

            # ---- reduce gathered rows: [P, G*D] -> [P, D] (sum over G)
            red = [sb.tile([P, D], F32, name=f"red{i}") for i in range(4)]
            for i in range(4):
                nc.vector.tensor_reduce(
                    out=red[i][:],
                    in_=gbuf[i][:].rearrange("p (g d) -> p d g", d=D),
                    axis=AX, op=OP.add)
            rel_sum = sb.tile([P, D], F32)
            ent_sum = sb.tile([P, D], F32)
            nc.vector.tensor_add(out=rel_sum[:], in0=red[0][:], in1=red[1][:])
            nc.vector.tensor_add(out=ent_sum[:], in0=red[2][:], in1=red[3][:])

            # ---- GCN: support = tanh((rel_sum@Wr' + ent_sum@We') + gcnb)
            relT = sb.tile([P, P], F32)
            transpose_to(relT[:], rel_sum[:], "rel")
            entT = sb.tile([P, P], F32)
            transpose_to(entT[:], ent_sum[:], "ent")
            sup_ps = ps.tile([P, D], F32, name="sup_ps", tag="mm")
            nc.tensor.matmul(out=sup_ps[:], lhsT=relT[:], rhs=w("wrT"),
                             start=True, stop=False)
            nc.tensor.matmul(out=sup_ps[:], lhsT=entT[:], rhs=w("weT"),
                             start=False, stop=True)
            support = sb.tile([P, D], F32)
            nc.vector.tensor_add(out=support[:], in0=sup_ps[:], in1=w("gcnb"))
            nc.scalar.activation(out=support[:], in_=support[:], func=ACT.Tanh)

            # ---- FFN + residual + layernorm -> support_g
            supT = sb.tile([P, P], F32)
            transpose_to(supT[:], support[:], "sup")
            hidT = []
            for j in range(2):
                hp = ps.tile([P, D], F32, name=f"hid_ps{j}", tag="mm")
                nc.tensor.matmul(out=hp[:], lhsT=w("p1wT")[:, j * D:(j + 1) * D],
                                 rhs=supT[:], start=True, stop=True)
                ht = sb.tile([P, P], F32, name=f"hidT{j}")
                nc.scalar.activation(out=ht[:], in_=hp[:], func=ACT.Relu,
                                     bias=w("p1b")[:, j:j + 1])
                hidT.append(ht)
            o2 = ps.tile([P, D], F32, name="o2", tag="mm")
            nc.tensor.matmul(out=o2[:], lhsT=hidT[0][:], rhs=w("p2wTa"),
                             start=True, stop=False)
            nc.tensor.matmul(out=o2[:], lhsT=hidT[1][:], rhs=w("p2wTb"),
                             start=False, stop=True)
            z = sb.tile([P, D], F32)
            nc.vector.tensor_add(out=z[:], in0=o2[:], in1=support[:])
            nc.vector.tensor_add(out=z[:], in0=z[:], in1=w("p2b"))
            # layernorm (unbiased std, eps added to std)
            zsum = sb.tile([P, 1], F32)
            nc.vector.tensor_reduce(out=zsum[:], in_=z[:], axis=AX, op=OP.add)
            zmean = sb.tile([P, 1], F32)
            nc.scalar.mul(out=zmean[:], in_=zsum[:], mul=1.0 / D)
            xc = sb.tile([P, D], F32)
            nc.vector.tensor_scalar(out=xc[:], in0=z[:], scalar1=zmean[:, 0:1],
                                    scalar2=None, op0=OP.subtract)
            sqt = sb.tile([P, D], F32)
            varsum = sb.tile([P, 1], F32)
            nc.scalar.activation(out=sqt[:], in_=xc[:], func=ACT.Square,
                                 accum_out=varsum[:])
            sigma = sb.tile([P, 1], F32)
            nc.scalar.activation(out=sigma[:], in_=varsum[:], func=ACT.Sqrt,
                                 scale=1.0 / (D - 1))
            nc.vector.tensor_scalar(out=sigma[:], in0=sigma[:], scalar1=1e-3,
                                    scalar2=None, op0=OP.add)
            rec = sb.tile([P, 1], F32)
            nc.vector.reciprocal(out=rec[:], in_=sigma[:])
            sg = sb.tile([P, D], F32)
            nc.vector.tensor_scalar(out=sg[:], in0=xc[:], scalar1=rec[:, 0:1],
                                    scalar2=None, op0=OP.mult)
            nc.vector.tensor_tensor(out=sg[:], in0=sg[:], in1=w("lna"),
                                    op=OP.mult)
            nc.vector.tensor_tensor(out=sg[:], in0=sg[:], in1=w("lnb"),
                                    op=OP.add)

            # ---- AllGather support_g -> (1024, D) on every core
            ag_in = dram.tile([P, D], F32)
            ag_out = dram.tile([NCORES * P, D], F32)
            nc.sync.dma_start(out=ag_in[:], in_=sg[:])
            nc.gpsimd.collective_compute(
                "AllGather", OP.bypass,
                replica_groups=[list(range(NCORES))],
                ins=[ag_in.opt()], outs=[ag_out.opt()])
            sg_all = sb.tile([P, NCORES, D], F32)
            nc.sync.dma_start(
                out=sg_all[:],
                in_=ag_out[:].rearrange("(c p) d -> p c d", c=NCORES))
            sgT = sb.tile([P, NCORES * P], F32)
            for c in range(NCORES):
                transpose_to(sgT[:, c * P:(c + 1) * P], sg_all[:, c, :], f"sg{c}")

            # ---- LSTM + attention (step-0 state precomputed above)
            gts = sb.tile([P, 8 * D], F32)
            rT_sb = sb.tile([P, P], F32)
            attn = sb.tile([P, NCORES * P], F32)
            rowsum = sb.tile([P, 1], F32)
            rsrec = sb.tile([P, 1], F32)

            for s in range(STEPS - 1):
                sc = ps.tile([P, NCORES * P], F32, name=f"sc{s}", tag="scores",
                             bufs=1)
                for j in range(2):
                    nc.tensor.matmul(out=sc[:, 512 * j:512 * (j + 1)],
                                     lhsT=hoT[:],
                                     rhs=sgT[:, 512 * j:512 * (j + 1)],
                                     start=True, stop=True)
                # softmax; exp without max-subtraction (|scores| <~ 60)
                nc.scalar.activation(out=attn[:], in_=sc[:], func=ACT.Exp,
                                     accum_out=rowsum[:])
                nc.vector.reciprocal(out=rsrec[:], in_=rowsum[:])
                nc.vector.tensor_scalar(out=attn[:], in0=attn[:],
                                        scalar1=rsrec[:, 0:1], scalar2=None,
                                        op0=OP.mult)
                rp = ps.tile([P, D], F32, name=f"rp{s}", tag="mm")
                for c in range(NCORES):
                    at = sb.tile([P, P], F32, name=f"at{s}{c}", tag="atT",
                                 bufs=2)
                    transpose_to(at[:], attn[:, c * P:(c + 1) * P], f"at{s}{c}")
                    nc.tensor.matmul(out=rp[:], lhsT=sg_all[:, c, :], rhs=at[:],
                                     start=(c == 0), stop=(c == NCORES - 1))
                nc.vector.tensor_copy(out=rT_sb[:], in_=rp[:])
                # gates for step s+1: gts_q + ho@WhhA' + r@WhhB'
                for j in range(2):
                    gp = ps.tile([P, 512], F32, name=f"g{s}{j}", tag="gates")
                    sl = slice(512 * j, 512 * (j + 1))
                    nc.tensor.matmul(out=gp[:], lhsT=hoT[:],
                                     rhs=w("whhTa")[:, sl],
                                     start=True, stop=False)
                    nc.tensor.matmul(out=gp[:], lhsT=rT_sb[:],
                                     rhs=w("whhTb")[:, sl],
                                     start=False, stop=True)
                    nc.vector.tensor_add(out=gts[:, sl], in0=gp[:],
                                         in1=gts_q[:, sl])
                nc.scalar.activation(out=si[:], in_=gts[:, 0:256],
                                     func=ACT.Sigmoid)
                nc.scalar.activation(out=tg[:], in_=gts[:, 512:768],
                                     func=ACT.Tanh)
                nc.scalar.activation(out=so[:], in_=gts[:, 768:896],
                                     func=ACT.Sigmoid)
                nc.scalar.activation(out=sf[:], in_=gts[:, 256:512],
                                     func=ACT.Sigmoid)
                nc.vector.tensor_tensor(out=sf[:], in0=sf[:], in1=c_sb[:],
                                        op=OP.mult)
                nc.vector.tensor_tensor(out=si[:], in0=si[:], in1=tg[:],
                                        op=OP.mult)
                nc.vector.tensor_add(out=c_sb[:], in0=sf[:], in1=si[:])
                nc.scalar.activation(out=tch[:], in_=c_sb[:, 0:D], func=ACT.Tanh)
                nc.vector.tensor_tensor(out=tch[:], in0=so[:], in1=tch[:],
                                        op=OP.mult)
                nc.vector.tensor_add(out=ho[:], in0=q_sb[:], in1=tch[:])
                if s < STEPS - 2:
                    transpose_to(hoT[:], ho[:], f"ho{s + 1}")

            # ---- cosine similarity against own support_g shard
            m1 = sb.tile([P, D], F32)
            nc.vector.tensor_tensor(out=m1[:], in0=ho[:], in1=sg[:], op=OP.mult)
            cross = sb.tile([P, 1], F32)
            nc.vector.tensor_reduce(out=cross[:], in_=m1[:], axis=AX, op=OP.add)
            n1 = sb.tile([P, 1], F32)
            n2 = sb.tile([P, 1], F32)
            nc.scalar.activation(out=m1[:], in_=ho[:], func=ACT.Square,
                                 accum_out=n1[:])
            nc.scalar.activation(out=m1[:], in_=sg[:], func=ACT.Square,
                                 accum_out=n2[:])
            nc.vector.tensor_tensor(out=n1[:], in0=n1[:], in1=n2[:], op=OP.mult)
            nc.scalar.activation(out=n1[:], in_=n1[:], func=ACT.Sqrt)
            nc.vector.reciprocal(out=n1[:], in_=n1[:])
            res = sb.tile([P, 1], F32)
            nc.vector.tensor_tensor(out=res[:], in0=cross[:], in1=n1[:],
                                    op=OP.mult)
            nc.sync.dma_start(out=out_d[:], in_=res[:])
    nc.compile()
    return nc


def _prep_inputs(relations, entities, query, symbol_emb, gcn_w_w, gcn_w_b,
                 p1_w, p1_b, p2_w, p2_b, ln_a, ln_b, w_ih, w_hh, b_ih, b_hh):
    f32 = np.float32
    table = np.ascontiguousarray(symbol_emb, dtype=f32)
    B = relations.shape[0]
    rel = np.asarray(relations).astype(np.int32)
    ent = np.asarray(entities).astype(np.int32)
    qry = np.asarray(query).astype(np.int32).reshape(B, 1)
    inv = f32(1.0 / B)                     # reference divides by B (quirk)
    wpack = np.empty((P, WCOLS), f32)

    def put(name, arr):
        c0, n = _WOFF[name]
        wpack[:, c0:c0 + n] = arr

    p2wT = np.asarray(p2_w).T.astype(f32)
    whhT = np.asarray(w_hh).T.astype(f32)
    put("wrT", (np.asarray(gcn_w_w)[:, :D] * inv).T)
    put("weT", (np.asarray(gcn_w_w)[:, D:] * inv).T)
    put("gcnb", np.broadcast_to(np.asarray(gcn_w_b) * (K / B), (P, D)))
    put("p1wT", np.asarray(p1_w).T)
    put("p1b", np.asarray(p1_b).reshape(2, P).T)
    put("p2wTa", p2wT[:D])
    put("p2wTb", p2wT[D:])
    put("p2b", np.broadcast_to(np.asarray(p2_b), (P, D)))
    put("lna", np.broadcast_to(np.asarray(ln_a), (P, D)))
    put("lnb", np.broadcast_to(np.asarray(ln_b), (P, D)))
    put("wihT", np.asarray(w_ih).T)
    put("whhTa", whhT[:D])
    put("whhTb", whhT[D:])
    put("gbias", np.broadcast_to(np.asarray(b_ih) + np.asarray(b_hh),
                                 (P, 8 * D)))
    in_maps = []
    for c in range(NCORES):
        rows = slice(c * P, (c + 1) * P)
        m = {
            "table": table, "wpack": wpack,
            "idx": np.ascontiguousarray(
                np.concatenate([rel[rows], ent[rows]], axis=1)),
            "qidx": np.ascontiguousarray(qry[rows]),
        }
        in_maps.append(m)
    return in_maps


def kernel(**inputs) -> np.ndarray:
    if "nc" not in _CACHE:
        _CACHE["nc"] = _build()
    nc = _CACHE["nc"]
    in_maps = _prep_inputs(**inputs)
    res = run_bass_kernel_spmd(nc, in_maps, list(range(NCORES)), trace=False)
    return np.concatenate([res.results[c]["out"][:, 0] for c in range(NCORES)])
